# revision 19
# baseline (speedup 1.0000x reference)
"""Category-specific linear (MoE-style routed batched matmul) on 8 trn2 cores.

out[b, s, h] = sum_i x[b, s, i] * W[cat_ids[b], i, h] + bias[cat_ids[b], h]

Shapes (hardcoded): x (32, 512, 1024) f32, cat_ids (32,) int, W (16, 1024, 4096)
f32, b (16, 4096) f32 -> out (32, 512, 4096) f32.

Strategy: data-parallel over batch, 4 batches per core, with host-side routing
that always packs one same-category PAIR of batches plus two singles per core
(slot capacities [2, 1, 1] batches). With 32 batches over 16 categories there
are always >= (32 - 16)/2 = 8 disjoint same-category pairs, so this packing is
feasible for ANY cat_ids. Each core then loads only 3 weight matrices (24 MB
fp16) instead of 4, keeping the kernel compute-bound.

Per core, over sections (slot, half) in [(0,0),(0,1),(1,0),(1,1),(2,0),(2,1)]:
  stream W[slot]-half as 8 tiles [128, 2048] (512 KB DMAs, sync ring),
  prefetched one full section ahead;
  for m over the slot's 128-sample tiles (8 for slot 0, 4 for 1/2):
    for kt(8) x n4(4): fp16 matmul -> psum[n4] (accumulate over kt)
    evict psum in 4 chunks: DVE add bias -> sbuf, DMA 256 KB to out

Scheduling notes (from trace analysis):
- HWDGE completion semaphores are 8 rotating lanes shared across ALL queues
  in the scheduler's global order: each dma_start waits for the 8th-previous
  DMA (any ring) to complete. Cold-start therefore keeps the competing DMA
  set tiny: 8 W tiles + 2 xt chunks + bias; later xt arrives in deferred
  2 x [128, 4, 512] chunks per batch, and each section's W tiles are emitted
  a section early so lane chaining never starves the weight stream.
- The very first m-tile pair runs kt-outer (interleaving two m-tiles per W
  tile) so cold-start W demand (1.73 us/tile) stays below DMA supply
  (1.54 us/tile); afterwards W is SBUF-resident and the loop reverts to
  m-outer for psum-bank pipelining.
- Warmup is ~6 fp16 matmuls: enough PE activity to open the HAM un-throttle
  window while the first DMAs land. fp16 (never fp32) keeps FWL enabled, so
  LDWEIGHTS hides behind the previous matmul and the steady-state issue
  interval is ~216 ns (512 moving rows @ 2.4 GHz + NX overhead).
- fp16 runs the PE at 1 cycle/row with ~3e-4 relative error.
"""

import numpy as np

import concourse.bacc as bacc
import concourse.mybir as mybir
import concourse.bass as bass
import concourse.tile as tile
from concourse.bass_utils import run_bass_kernel_spmd

N_CORES = 8
B, S, K, H = 32, 512, 1024, 4096
BPC = B // N_CORES          # batches per core
P = 128                     # partitions
KT = K // P                 # k tiles (8)
MT = S // P                 # sample tiles per batch (4)
NHALF = 2                   # n halves
NH = H // NHALF             # cols per half (2048)
NMM = NH // 512             # 512-wide matmuls per half (4)
XC = 2                      # xt chunks per batch
XKT = KT // XC              # k tiles per xt chunk (4)
SLOT_BATCHES = (2, 1, 1)    # batches per weight slot
NSLOT = len(SLOT_BATCHES)
N_WARM = 6                  # fp16 warmup matmuls (HAM un-throttle)

_COMPILED = None


def _build():
    nc = bacc.Bacc("TRN2", target_bir_lowering=False, debug=False)
    f32 = mybir.dt.float32
    f16 = mybir.dt.float16

    xt_ap = nc.dram_tensor("xt", [BPC, K, S], f16, kind="ExternalInput").ap()
    w_ap = nc.dram_tensor("w", [NSLOT, K, H], f16, kind="ExternalInput").ap()
    bias_ap = nc.dram_tensor("bias", [NSLOT, H], f32, kind="ExternalInput").ap()
    out_ap = nc.dram_tensor("out", [BPC, S, H], f32, kind="ExternalOutput").ap()

    sections = [(s, h) for s in range(NSLOT) for h in range(NHALF)]
    slot_base = [sum(SLOT_BATCHES[:s]) for s in range(NSLOT)]

    with tile.TileContext(nc) as tc:
        with (
            tc.tile_pool(name="xt_pool", bufs=BPC * XC + 1) as xt_pool,
            tc.tile_pool(name="w_pool", bufs=24) as w_pool,
            tc.tile_pool(name="bias_pool", bufs=2) as bias_pool,
            tc.tile_pool(name="out_pool", bufs=16) as out_pool,
            tc.tile_pool(name="ps_pool", bufs=8, space="PSUM") as ps_pool,
        ):
            # Short fp16 warmup: keeps the PE busy while the first DMAs land
            # and opens the HAM un-throttle window. Result read once so DCE
            # keeps it.
            warm_x = xt_pool.tile([P, P], f16, name="warm_x", tag="warm")
            warm_w = w_pool.tile([P, 512], f16, tag="w", name="warm_w")
            nc.vector.memset(warm_x[:], 0.0)
            nc.vector.memset(warm_w[:], 0.0)
            warm_ps = ps_pool.tile([P, 512], f32, tag="ps", name="warm_ps")
            for _ in range(N_WARM):
                nc.tensor.matmul(
                    warm_ps[:], warm_x[:], warm_w[:], start=True, stop=True,
                    skip_group_check=True,
                )
            warm_out = out_pool.tile([P, 4], f32, name="warm_out", tag="warmo")
            nc.vector.tensor_copy(warm_out[:], warm_ps[:, 0:4])

            def evict(ps, bias_t, b, mm, half):
                """psum -> (+bias on DVE) -> sbuf -> out, in 4 512-col chunks."""
                for n4 in range(NMM):
                    out_t = out_pool.tile([P, 512], f32)
                    nc.vector.tensor_add(
                        out_t[:], ps[n4][:],
                        bias_t[:, n4 * 512 : (n4 + 1) * 512],
                    )
                    nc.scalar.dma_start(
                        out_ap[
                            b,
                            mm * P : (mm + 1) * P,
                            half * NH + n4 * 512 : half * NH + (n4 + 1) * 512,
                        ],
                        out_t[:],
                    )

            def fetch_w(sec, kt):
                s, half = sec
                w_r = w_ap[s].rearrange("(kt p) n -> p kt n", p=P)
                w_t = w_pool.tile([P, NH], f16, tag="w", name="w_t")
                nc.sync.dma_start(w_t[:], w_r[:, kt, half * NH : (half + 1) * NH])
                return w_t

            def fetch_bias(sec, gate=False):
                s, half = sec
                bias_t = bias_pool.tile([P, NH], f32, name="bias_t")
                if gate:
                    # WAW marker: the DMA must wait for this DVE memset,
                    # which (by DVE program order) runs only after the cold
                    # window — keeps the scheduler from hoisting the
                    # transfer into the bandwidth-limited start.
                    nc.vector.memset(bias_t[:, 0:8], 0.0)
                bias_src = bias_ap[s, half * NH : (half + 1) * NH]
                nc.gpsimd.dma_start(
                    out=bias_t[:],
                    in_=bass.AP(
                        tensor=bias_src.tensor,
                        offset=bias_src.offset,
                        ap=[[0, P]] + list(bias_src.ap),
                    ),
                )
                return bias_t

            def fetch_w_bias(sec):
                """Issue the W tiles + bias for section (slot, half)."""
                return [fetch_w(sec, kt) for kt in range(KT)], fetch_bias(sec)

            xt_ts = {}  # global batch index -> [XC chunk tiles]

            def ensure_xt(gb, gate=False):
                # xt chunks [128, XKT, 512] on the scalar ring. gate=True
                # adds a WAW marker memset so the DMA cannot be hoisted into
                # the bandwidth-limited cold window (it would steal DMA
                # bandwidth from the critical W stream).
                if gb in xt_ts:
                    return
                chunks = []
                for c in range(XC):
                    xt_t = xt_pool.tile([P, XKT, S], f16, name="xt_t", tag="xt")
                    if gate:
                        nc.vector.memset(xt_t[:, 0, 0:8], 0.0)
                    nc.scalar.dma_start(
                        xt_t[:],
                        xt_ap[gb, c * XKT * P : (c + 1) * XKT * P, :].rearrange(
                            "(kt p) m -> p kt m", p=P
                        ),
                    )
                    chunks.append(xt_t)
                xt_ts[gb] = chunks

            def lhsT_of(gb, kt, mm):
                c, ktl = divmod(kt, XKT)
                return xt_ts[gb][c][:, ktl, mm * P : (mm + 1) * P]

            # Section 0's W owns the head of the HWDGE lane rotation
            # (cold-start critical); later sections prefetch one ahead at
            # normal priority. The scheduler round-robins dependency-free
            # DMA issues across engine queues, so W and xt share the cold
            # window roughly fairly; the kt-outer pair below tolerates that.
            with tc.high_priority():
                cur_w, cur_bias = fetch_w_bias(sections[0])
            ensure_xt(0)

            for si, (s, half) in enumerate(sections):
                nb = SLOT_BATCHES[s]
                bi0 = slot_base[s]
                w_tiles, bias_t = cur_w, cur_bias
                if 0 < si < len(sections) - 1:
                    cur_w, cur_bias = fetch_w_bias(sections[si + 1])

                m0 = 0
                if si == 0:
                    # Cold start: kt-outer over the first m-PAIR so each
                    # W tile feeds 8 matmuls (~1.73 us) vs its ~1.54 us
                    # DMA, keeping the PE fed while W streams in.
                    ps2 = [
                        [
                            ps_pool.tile([P, 512], f32, tag="ps", name="ps")
                            for _ in range(NMM)
                        ]
                        for _ in range(2)
                    ]
                    for kt in range(KT):
                        for mi in range(2):
                            lhsT = lhsT_of(bi0, kt, mi)
                            for n4 in range(NMM):
                                nc.tensor.matmul(
                                    ps2[mi][n4][:],
                                    lhsT,
                                    w_tiles[kt][:, n4 * 512 : (n4 + 1) * 512],
                                    start=(kt == 0),
                                    stop=(kt == KT - 1),
                                )
                            if kt == KT - 1:
                                # evict mi's banks right away so the next
                                # m-tile's matmuls get psum banks sooner
                                evict(ps2[mi], bias_t, bi0, mi, half)
                    m0 = 2
                    # Section 1's prefetch is emitted only now, with a gated
                    # bias, so none of it competes with the cold window.
                    cur_w = [fetch_w(sections[1], kt) for kt in range(KT)]
                    cur_bias = fetch_bias(sections[1], gate=True)

                for m in range(m0, nb * MT):
                    b, mm = divmod(m, MT)
                    if half == 0 and b + 1 < nb and m == m0:
                        # start the next batch's xt stream with lead time
                        ensure_xt(bi0 + b + 1, gate=True)
                    if half == 0 and s + 1 < NSLOT and m == m0 + 1:
                        # and the next slot's first batch
                        ensure_xt(slot_base[s + 1], gate=True)
                    ps = [
                        ps_pool.tile([P, 512], f32, tag="ps", name="ps")
                        for _ in range(NMM)
                    ]
                    for kt in range(KT):
                        lhsT = lhsT_of(bi0 + b, kt, mm)
                        for n4 in range(NMM):
                            nc.tensor.matmul(
                                ps[n4][:],
                                lhsT,
                                w_tiles[kt][:, n4 * 512 : (n4 + 1) * 512],
                                start=(kt == 0),
                                stop=(kt == KT - 1),
                            )
                    evict(ps, bias_t, bi0 + b, mm, half)
    nc.compile()
    return nc


def _get_compiled():
    global _COMPILED
    if _COMPILED is None:
        _COMPILED = _build()
    return _COMPILED


def _pack(cat_ids):
    """Assign batches to cores with slot capacities [2,1,1] per core.

    Returns per-core (idx, slot_cats): idx = 4 batch indices ordered
    [pair0, pair1, single_b, single_c]; slot_cats = categories for the 3 slots.
    Always feasible: #disjoint same-cat pairs = (32 - #odd-count cats)/2 >= 8.
    """
    cat_ids = np.asarray(cat_ids)
    by_cat = {}
    for i, c in enumerate(cat_ids.tolist()):
        by_cat.setdefault(c, []).append(i)
    pairs = []
    singles = []
    for c, idxs in sorted(by_cat.items()):
        n = len(idxs)
        for j in range(n // 2):
            pairs.append((c, idxs[2 * j], idxs[2 * j + 1]))
        if n % 2:
            singles.append((c, idxs[-1]))
    assert len(pairs) >= N_CORES, "impossible: <8 same-cat pairs among 32 batches"
    core_pairs = pairs[:N_CORES]
    # leftovers: extra pairs flatten into singles
    for c, i, j in pairs[N_CORES:]:
        singles.append((c, i))
        singles.append((c, j))
    assert len(singles) == 2 * N_CORES
    cores = []
    for ci in range(N_CORES):
        c, i, j = core_pairs[ci]
        (cb, ib), (cc, ic) = singles[2 * ci], singles[2 * ci + 1]
        cores.append(([i, j, ib, ic], [c, cb, cc]))
    return cores


def run_sharded(x, cat_ids, W, b, trace=False, **spmd_kwargs):
    """Shard, run on 8 cores, unshard. Returns (out, BassKernelResults)."""
    x = np.ascontiguousarray(np.asarray(x), dtype=np.float32)
    cat_ids = np.asarray(cat_ids).astype(np.int64)
    W = np.ascontiguousarray(np.asarray(W), dtype=np.float32)
    b = np.ascontiguousarray(np.asarray(b), dtype=np.float32)

    nc = _get_compiled()
    cores = _pack(cat_ids)

    in_maps = []
    for idx, slot_cats in cores:
        in_maps.append(
            {
                "xt": np.ascontiguousarray(x[idx].transpose(0, 2, 1).astype(np.float16)),
                "w": np.ascontiguousarray(W[slot_cats].astype(np.float16)),
                "bias": np.ascontiguousarray(b[slot_cats]),
            }
        )

    res = run_bass_kernel_spmd(
        nc, in_maps, list(range(N_CORES)), trace=trace, **spmd_kwargs
    )

    out = np.empty((B, S, H), dtype=np.float32)
    for c, (idx, _) in enumerate(cores):
        out[idx] = res.results[c]["out"]
    return out, res


def kernel(x, cat_ids, W, b):
    out, _ = run_sharded(x, cat_ids, W, b)
    return out


# revision 21
# speedup vs baseline: 1.0442x; 1.0442x over previous
"""Category-specific linear (MoE-style routed batched matmul) on 8 trn2 cores.

out[b, s, h] = sum_i x[b, s, i] * W[cat_ids[b], i, h] + bias[cat_ids[b], h]

Shapes (hardcoded): x (32, 512, 1024) f32, cat_ids (32,) int, W (16, 1024, 4096)
f32, b (16, 4096) f32 -> out (32, 512, 4096) f32.

Strategy: data-parallel over batch, 4 batches per core, with host-side routing
that always packs one same-category PAIR of batches plus two singles per core
(slot capacities [2, 1, 1] batches). With 32 batches over 16 categories there
are always >= (32 - 16)/2 = 8 disjoint same-category pairs, so this packing is
feasible for ANY cat_ids. Each core then loads only 3 weight matrices (24 MB
fp16) instead of 4, keeping the kernel compute-bound.

Per core, over sections (slot, half) in [(0,0),(0,1),(1,0),(1,1),(2,0),(2,1)]:
  stream W[slot]-half as 8 tiles [128, 2048] (512 KB DMAs, sync ring),
  prefetched one full section ahead;
  for m over the slot's 128-sample tiles (8 for slot 0, 4 for 1/2):
    for kt(8) x n4(4): fp16 matmul -> psum[n4] (accumulate over kt)
    evict psum in 4 chunks: DVE add bias -> sbuf, DMA 256 KB to out

Scheduling notes (from trace analysis):
- HWDGE completion semaphores are 8 rotating lanes shared across ALL queues
  in the scheduler's global order: each dma_start waits for the 8th-previous
  DMA (any ring) to complete. Cold-start therefore keeps the competing DMA
  set tiny: 8 W tiles + 2 xt chunks + bias; later xt arrives in deferred
  2 x [128, 4, 512] chunks per batch, and each section's W tiles are emitted
  a section early so lane chaining never starves the weight stream.
- The very first m-tile pair runs kt-outer (interleaving two m-tiles per W
  tile) so cold-start W demand (1.73 us/tile) stays below DMA supply
  (1.54 us/tile); afterwards W is SBUF-resident and the loop reverts to
  m-outer for psum-bank pipelining.
- Warmup is ~6 fp16 matmuls: enough PE activity to open the HAM un-throttle
  window while the first DMAs land. fp16 (never fp32) keeps FWL enabled, so
  LDWEIGHTS hides behind the previous matmul and the steady-state issue
  interval is ~216 ns (512 moving rows @ 2.4 GHz + NX overhead).
- fp16 runs the PE at 1 cycle/row with ~3e-4 relative error.
"""

import numpy as np

import concourse.bacc as bacc
import concourse.mybir as mybir
import concourse.bass as bass
import concourse.tile as tile
from concourse.bass_utils import run_bass_kernel_spmd

N_CORES = 8
B, S, K, H = 32, 512, 1024, 4096
BPC = B // N_CORES          # batches per core
P = 128                     # partitions
KT = K // P                 # k tiles (8)
MT = S // P                 # sample tiles per batch (4)
NSEC = 4                    # n quarter-sections per slot
NH = H // NSEC              # cols per quarter (1024)
NMM = NH // 512             # 512-wide matmuls per quarter (2)
XC = 4                      # xt chunks per batch
XKT = KT // XC              # k tiles per xt chunk (2)
SLOT_BATCHES = (2, 1, 1)    # batches per weight slot
NSLOT = len(SLOT_BATCHES)
N_WARM = 6                  # fp16 warmup matmuls (HAM un-throttle)

_COMPILED = None


def _build():
    nc = bacc.Bacc("TRN2", target_bir_lowering=False, debug=False)
    f32 = mybir.dt.float32
    f16 = mybir.dt.float16

    xt_ap = nc.dram_tensor("xt", [BPC, K, S], f16, kind="ExternalInput").ap()
    w_ap = nc.dram_tensor("w", [NSLOT, K, H], f16, kind="ExternalInput").ap()
    bias_ap = nc.dram_tensor("bias", [NSLOT, H], f32, kind="ExternalInput").ap()
    out_ap = nc.dram_tensor("out", [BPC, S, H], f32, kind="ExternalOutput").ap()

    sections = [(s, q) for s in range(NSLOT) for q in range(NSEC)]
    slot_base = [sum(SLOT_BATCHES[:s]) for s in range(NSLOT)]

    with tile.TileContext(nc) as tc:
        with (
            tc.tile_pool(name="xt_pool", bufs=BPC * XC + 1) as xt_pool,
            tc.tile_pool(name="w_pool", bufs=24) as w_pool,
            tc.tile_pool(name="bias_pool", bufs=2) as bias_pool,
            tc.tile_pool(name="out_pool", bufs=16) as out_pool,
            tc.tile_pool(name="ps_pool", bufs=8, space="PSUM") as ps_pool,
        ):
            # Short fp16 warmup: keeps the PE busy while the first DMAs land
            # and opens the HAM un-throttle window. Result read once so DCE
            # keeps it.
            warm_x = xt_pool.tile([P, P], f16, name="warm_x", tag="warm")
            warm_w = w_pool.tile([P, 512], f16, tag="w", name="warm_w")
            nc.vector.memset(warm_x[:], 0.0)
            nc.vector.memset(warm_w[:], 0.0)
            warm_ps = ps_pool.tile([P, 512], f32, tag="ps", name="warm_ps")
            for _ in range(N_WARM):
                nc.tensor.matmul(
                    warm_ps[:], warm_x[:], warm_w[:], start=True, stop=True,
                    skip_group_check=True,
                )
            warm_out = out_pool.tile([P, 4], f32, name="warm_out", tag="warmo")
            nc.vector.tensor_copy(warm_out[:], warm_ps[:, 0:4])

            def evict(ps, bias_t, b, mm, half):
                """psum -> (+bias on DVE) -> sbuf -> out, in 4 512-col chunks."""
                for n4 in range(NMM):
                    out_t = out_pool.tile([P, 512], f32)
                    nc.vector.tensor_add(
                        out_t[:], ps[n4][:],
                        bias_t[:, n4 * 512 : (n4 + 1) * 512],
                    )
                    nc.scalar.dma_start(
                        out_ap[
                            b,
                            mm * P : (mm + 1) * P,
                            half * NH + n4 * 512 : half * NH + (n4 + 1) * 512,
                        ],
                        out_t[:],
                    )

            def fetch_w(sec, kt):
                s, half = sec
                w_r = w_ap[s].rearrange("(kt p) n -> p kt n", p=P)
                w_t = w_pool.tile([P, NH], f16, tag="w", name="w_t")
                nc.sync.dma_start(w_t[:], w_r[:, kt, half * NH : (half + 1) * NH])
                return w_t

            def fetch_bias(sec, gate=False):
                s, half = sec
                bias_t = bias_pool.tile([P, NH], f32, name="bias_t")
                if gate:
                    # WAW marker: the DMA must wait for this DVE memset,
                    # which (by DVE program order) runs only after the cold
                    # window — keeps the scheduler from hoisting the
                    # transfer into the bandwidth-limited start.
                    nc.vector.memset(bias_t[:, 0:8], 0.0)
                bias_src = bias_ap[s, half * NH : (half + 1) * NH]
                nc.gpsimd.dma_start(
                    out=bias_t[:],
                    in_=bass.AP(
                        tensor=bias_src.tensor,
                        offset=bias_src.offset,
                        ap=[[0, P]] + list(bias_src.ap),
                    ),
                )
                return bias_t

            def fetch_w_bias(sec):
                """Issue the W tiles + bias for section (slot, half)."""
                return [fetch_w(sec, kt) for kt in range(KT)], fetch_bias(sec)

            xt_ts = {}  # global batch index -> [XC chunk tiles]

            def ensure_xt(gb, gate=False):
                # xt chunks [128, XKT, 512] on the scalar ring. gate=True
                # adds a WAW marker memset so the DMA cannot be hoisted into
                # the bandwidth-limited cold window (it would steal DMA
                # bandwidth from the critical W stream).
                if gb in xt_ts:
                    return
                chunks = []
                for c in range(XC):
                    xt_t = xt_pool.tile([P, XKT, S], f16, name="xt_t", tag="xt")
                    if gate:
                        nc.vector.memset(xt_t[:, 0, 0:8], 0.0)
                    nc.scalar.dma_start(
                        xt_t[:],
                        xt_ap[gb, c * XKT * P : (c + 1) * XKT * P, :].rearrange(
                            "(kt p) m -> p kt m", p=P
                        ),
                    )
                    chunks.append(xt_t)
                xt_ts[gb] = chunks

            def lhsT_of(gb, kt, mm):
                c, ktl = divmod(kt, XKT)
                return xt_ts[gb][c][:, ktl, mm * P : (mm + 1) * P]

            # Section 0's W owns the head of the HWDGE lane rotation
            # (cold-start critical); later sections prefetch one ahead at
            # normal priority. The scheduler round-robins dependency-free
            # DMA issues across engine queues, so W and xt share the cold
            # window roughly fairly; the kt-outer pair below tolerates that.
            with tc.high_priority():
                cur_w, cur_bias = fetch_w_bias(sections[0])
            ensure_xt(0)

            for si, (s, half) in enumerate(sections):
                nb = SLOT_BATCHES[s]
                bi0 = slot_base[s]
                w_tiles, bias_t = cur_w, cur_bias
                if 0 < si < len(sections) - 1:
                    cur_w, cur_bias = fetch_w_bias(sections[si + 1])

                m0 = 0
                if si == 0:
                    # Cold start: kt-outer over a QUAD of m-tiles (4 m x 2
                    # psum banks) so each 256 KB W granule feeds 8 matmuls
                    # (~1.73 us) vs its ~0.8 us DMA -- 2x margin against the
                    # shared cold-start bandwidth.
                    ps2 = [
                        [
                            ps_pool.tile([P, 512], f32, tag="ps", name="ps")
                            for _ in range(NMM)
                        ]
                        for _ in range(4)
                    ]
                    for kt in range(KT):
                        for mi in range(4):
                            lhsT = lhsT_of(bi0, kt, mi)
                            for n4 in range(NMM):
                                nc.tensor.matmul(
                                    ps2[mi][n4][:],
                                    lhsT,
                                    w_tiles[kt][:, n4 * 512 : (n4 + 1) * 512],
                                    start=(kt == 0),
                                    stop=(kt == KT - 1),
                                )
                            if kt == KT - 1:
                                # evict mi's banks right away so the next
                                # m-tile's matmuls get psum banks sooner
                                evict(ps2[mi], bias_t, bi0, mi, half)
                    m0 = 4
                    # Batch 1's xt streams during the quad (the quad's 2x
                    # demand margin tolerates the bandwidth sharing) so it is
                    # ready when m=4 starts right after.
                    ensure_xt(bi0 + 1)
                    # Section 1's prefetch is emitted only now, with a gated
                    # bias, so none of it competes with the cold window.
                    cur_w = [fetch_w(sections[1], kt) for kt in range(KT)]
                    cur_bias = fetch_bias(sections[1], gate=True)

                for m in range(m0, nb * MT):
                    b, mm = divmod(m, MT)
                    if half == 0 and b + 1 < nb and m == m0:
                        # start the next batch's xt stream with lead time
                        ensure_xt(bi0 + b + 1, gate=True)
                    if half == 0 and s + 1 < NSLOT and m == m0 + 1:
                        # and the next slot's first batch
                        ensure_xt(slot_base[s + 1], gate=True)
                    ps = [
                        ps_pool.tile([P, 512], f32, tag="ps", name="ps")
                        for _ in range(NMM)
                    ]
                    for kt in range(KT):
                        lhsT = lhsT_of(bi0 + b, kt, mm)
                        for n4 in range(NMM):
                            nc.tensor.matmul(
                                ps[n4][:],
                                lhsT,
                                w_tiles[kt][:, n4 * 512 : (n4 + 1) * 512],
                                start=(kt == 0),
                                stop=(kt == KT - 1),
                            )
                    evict(ps, bias_t, bi0 + b, mm, half)
    nc.compile()
    return nc


def _get_compiled():
    global _COMPILED
    if _COMPILED is None:
        _COMPILED = _build()
    return _COMPILED


def _pack(cat_ids):
    """Assign batches to cores with slot capacities [2,1,1] per core.

    Returns per-core (idx, slot_cats): idx = 4 batch indices ordered
    [pair0, pair1, single_b, single_c]; slot_cats = categories for the 3 slots.
    Always feasible: #disjoint same-cat pairs = (32 - #odd-count cats)/2 >= 8.
    """
    cat_ids = np.asarray(cat_ids)
    by_cat = {}
    for i, c in enumerate(cat_ids.tolist()):
        by_cat.setdefault(c, []).append(i)
    pairs = []
    singles = []
    for c, idxs in sorted(by_cat.items()):
        n = len(idxs)
        for j in range(n // 2):
            pairs.append((c, idxs[2 * j], idxs[2 * j + 1]))
        if n % 2:
            singles.append((c, idxs[-1]))
    assert len(pairs) >= N_CORES, "impossible: <8 same-cat pairs among 32 batches"
    core_pairs = pairs[:N_CORES]
    # leftovers: extra pairs flatten into singles
    for c, i, j in pairs[N_CORES:]:
        singles.append((c, i))
        singles.append((c, j))
    assert len(singles) == 2 * N_CORES
    cores = []
    for ci in range(N_CORES):
        c, i, j = core_pairs[ci]
        (cb, ib), (cc, ic) = singles[2 * ci], singles[2 * ci + 1]
        cores.append(([i, j, ib, ic], [c, cb, cc]))
    return cores


def run_sharded(x, cat_ids, W, b, trace=False, **spmd_kwargs):
    """Shard, run on 8 cores, unshard. Returns (out, BassKernelResults)."""
    x = np.ascontiguousarray(np.asarray(x), dtype=np.float32)
    cat_ids = np.asarray(cat_ids).astype(np.int64)
    W = np.ascontiguousarray(np.asarray(W), dtype=np.float32)
    b = np.ascontiguousarray(np.asarray(b), dtype=np.float32)

    nc = _get_compiled()
    cores = _pack(cat_ids)

    in_maps = []
    for idx, slot_cats in cores:
        in_maps.append(
            {
                "xt": np.ascontiguousarray(x[idx].transpose(0, 2, 1).astype(np.float16)),
                "w": np.ascontiguousarray(W[slot_cats].astype(np.float16)),
                "bias": np.ascontiguousarray(b[slot_cats]),
            }
        )

    res = run_bass_kernel_spmd(
        nc, in_maps, list(range(N_CORES)), trace=trace, **spmd_kwargs
    )

    out = np.empty((B, S, H), dtype=np.float32)
    for c, (idx, _) in enumerate(cores):
        out[idx] = res.results[c]["out"]
    return out, res


def kernel(x, cat_ids, W, b):
    out, _ = run_sharded(x, cat_ids, W, b)
    return out
